# revision 13
# baseline (speedup 1.0000x reference)
"""Trainium2 Bass kernel for the DCM sparse-attention problem.

Math restructure: with t-hat/v-hat the row-normalized features and
S[(a,t),(b,v)] = <t-hat[a,t], v-hat[b,v]> the raw cosine logits, every
softmax-weighted aggregation in the reference collapses onto S:

  t2v[a,b,t] = sum_v vps1 * S            (free-dim group reduce)
  v2t[a,b,v] = sum_t tps1 * S            (indicator matmul over t)
  out[a,b]   = sum_t tps2[t] sum_v vps2[v] S[t,v]   (J trick + group reduce)

so the [A,B,T,D] intermediates never exist. Each of the 8 cores handles
8 of the 64 text rows (A-sharding, video replicated); no collectives.
"""

import sys

sys.path.insert(0, "/opt/trn_rl_repo")

import numpy as np

import concourse.bass as bass
import concourse.bacc as bacc
import concourse.tile as tile
from concourse import mybir
from concourse.bass_utils import run_bass_kernel_spmd

TAU = 100.0
A, T, B, V, D = 64, 32, 64, 12, 512
NCORES = 8
AL = A // NCORES          # a's per core = 8
AT = AL * T               # (a,t) rows per core = 256
BV = B * V                # (b,v) cols = 768
NMT = AT // 128           # M-tiles over (a,t) = 2
NKT = D // 128            # K-tiles over d = 4
APB = 128 // T            # a's per M-tile = 4
F32 = mybir.dt.float32
EXP = mybir.ActivationFunctionType.Exp
SQUARE = mybir.ActivationFunctionType.Square
SQRT = mybir.ActivationFunctionType.Sqrt
MUL = mybir.AluOpType.mult
X = mybir.AxisListType.X
NSL = [(0, 512), (512, 768)]                   # bank-aligned slices of 768
NSL3 = [(0, 512), (512, 1024), (1024, 1536)]   # ... of 1536
NSLJ = [(0, 512), (512, 832)]                  # ... of 832


def _build_program():
    nc = bacc.Bacc("TRN2", target_bir_lowering=False)

    tT_d = nc.declare_dram_parameter("tT", [D, AT], F32, isOutput=False)
    vT_d = nc.declare_dram_parameter("vT", [D, BV], F32, isOutput=False)
    mask_d = nc.declare_dram_parameter("mask", [AT, 1], F32, isOutput=False)
    ident_d = nc.declare_dram_parameter("ident", [128, 128], F32, isOutput=False)
    ind36_d = nc.declare_dram_parameter("ind36", [128, 2 * 36], F32, isOutput=False)
    onesc_d = nc.declare_dram_parameter("onesc", [128, 1], F32, isOutput=False)
    out_d = nc.declare_dram_parameter("out", [AL, B], F32, isOutput=True)

    with tile.TileContext(nc) as tc:
        with (
            tc.tile_pool(name="consts", bufs=1) as consts,
            tc.tile_pool(name="inputs", bufs=1) as inputs,
            tc.tile_pool(name="sq", bufs=3) as sqp,
            tc.tile_pool(name="big", bufs=1) as bigp,
            tc.tile_pool(name="smalls", bufs=1) as smalls,
            tc.tile_pool(name="psA", bufs=1, space="PSUM") as psA,
            tc.tile_pool(name="psB", bufs=1, space="PSUM") as psB,
        ):
            # ---- constants / small inputs ----
            ident = consts.tile([128, 128], F32)
            nc.sync.dma_start(out=ident, in_=ident_d[:, :])
            ind36 = consts.tile([128, 2 * 36], F32)
            nc.sync.dma_start(out=ind36, in_=ind36_d[:, :])
            onesc = consts.tile([128, 1], F32)
            nc.sync.dma_start(out=onesc, in_=onesc_d[:, :])
            maskt = [consts.tile([128, 1], F32, name=f"maskt{i}") for i in range(NMT)]
            tau_m = [consts.tile([128, 1], F32, name=f"tau_m{i}") for i in range(NMT)]
            for i in range(NMT):
                nc.sync.dma_start(out=maskt[i], in_=mask_d[128 * i:128 * (i + 1), :])
                nc.vector.tensor_scalar_mul(tau_m[i], maskt[i], TAU)

            # ---- main inputs ([d, row] layouts); vT first (longest chain) ----
            vT = [inputs.tile([128, BV], F32, name=f"vT{k}") for k in range(NKT)]
            tT = [inputs.tile([128, AT], F32, name=f"tT{k}") for k in range(NKT)]
            for k in range(NKT):
                nc.sync.dma_start(out=vT[k], in_=vT_d[128 * k:128 * (k + 1), :])
            for k in range(NKT):
                nc.sync.dma_start(out=tT[k], in_=tT_d[128 * k:128 * (k + 1), :])

            # ---- norms: ACT squares + ones-matmul column sums ----
            ps_ssv = psB.tile([1, BV], F32, tag="j")
            ps_sst = psB.tile([1, AT], F32, tag="j")
            for k in range(NKT):
                sqv = sqp.tile([128, BV], F32, tag="sq", name=f"sqv{k}")
                nc.scalar.activation(sqv, vT[k], SQUARE)
                for lo, hi in NSL:
                    nc.tensor.matmul(ps_ssv[:, lo:hi], onesc, sqv[:, lo:hi],
                                     start=(k == 0), stop=(k == NKT - 1))
            for k in range(NKT):
                sqt = sqp.tile([128, AT], F32, tag="sqt", name=f"sqt{k}")
                nc.scalar.activation(sqt, tT[k], SQUARE)
                nc.tensor.matmul(ps_sst, onesc, sqt,
                                 start=(k == 0), stop=(k == NKT - 1))

            # rv chain: sqrt (skinny) -> broadcast -> wide approx reciprocal
            nv_row = smalls.tile([1, BV], F32)
            nc.scalar.activation(nv_row, ps_ssv, SQRT)
            nv_bc = bigp.tile([128, BV], F32)
            nc.gpsimd.partition_broadcast(nv_bc, nv_row, channels=128)
            rv_bc = bigp.tile([128, BV], F32)
            nc.vector.reciprocal_approx_fast(rv_bc, nv_bc)

            # r_t: sqrt of norm row, transpose to per-partition column, recip
            r_t = [smalls.tile([128, 1], F32, name=f"r_t{i}") for i in range(NMT)]
            nt_row = smalls.tile([1, AT], F32)
            nc.scalar.activation(nt_row, ps_sst, SQRT)
            for i in range(NMT):
                ps_tr = psB.tile([128, 1], F32, tag="j", name=f"ps_tr{i}")
                nc.tensor.transpose(ps_tr, nt_row[:, 128 * i:128 * (i + 1)],
                                    ident[0:1, 0:1])
                nc.vector.reciprocal_approx_fast(r_t[i], ps_tr)

            # ---- S matmuls + downstream per M-tile ----
            esf = [bigp.tile([128, 832], F32, name=f"esf{i}") for i in range(NMT)]
            rhs_v = [bigp.tile([128, 2 * BV], F32, name=f"rhs_v{i}")
                     for i in range(NMT)]
            for i in range(NMT):
                ps_s = psA.tile([128, BV], F32, tag="s")
                for lo, hi in NSL:
                    for k in range(NKT):
                        nc.tensor.matmul(
                            ps_s[:, lo:hi],
                            tT[k][:, 128 * i:128 * (i + 1)],
                            vT[k][:, lo:hi],
                            start=(k == 0), stop=(k == NKT - 1))
                # Sp = ps_s * r_t (per-row) * rv (per-col)
                sp = bigp.tile([128, BV], F32, name=f"sp{i}")
                nc.vector.scalar_tensor_tensor(sp, ps_s, r_t[i], rv_bc,
                                               op0=MUL, op1=MUL)
                # big = [ES | E]; E = exp(tau*m*Sp)
                big = bigp.tile([128, 2 * BV], F32, name=f"big{i}")
                nc.scalar.activation(big[:, BV:], sp, EXP, scale=tau_m[i][:, :])
                nc.vector.tensor_tensor(big[:, :BV], big[:, BV:], sp, op=MUL)
                # t2v = groupsum(ES)/groupsum(E); E3 = exp(tau*t2v)
                red = smalls.tile([128, 128], F32, name=f"red{i}")
                nc.vector.reduce_sum(red, big.rearrange("p (g v) -> p g v", v=V),
                                     axis=X)
                rdn = smalls.tile([128, B], F32, name=f"rdn{i}")
                nc.vector.reciprocal_approx_fast(rdn, red[:, B:])
                t2v = smalls.tile([128, B], F32, name=f"t2v{i}")
                nc.vector.tensor_tensor(t2v, red[:, :B], rdn, op=MUL)
                nc.scalar.activation(esf[i][:, BV:], t2v, EXP, scale=TAU)
                # ES3 = Sp * E3 (E3 broadcast over v via step-0 AP)
                e3sl = esf[i][:, BV:BV + B]
                e3b = bass.AP(tensor=e3sl.tensor, offset=e3sl.offset,
                              ap=[e3sl.ap[0], e3sl.ap[1], [0, V]])
                nc.vector.tensor_tensor(esf[i][:, :BV], sp, e3b, op=MUL)
                # rhs_v = [E2S | E2] = mask * [ES | E]  (gpsimd, line-rate TS)
                nc.gpsimd.tensor_scalar_mul(rhs_v[i][:, :BV], big[:, :BV],
                                            maskt[i])
                nc.gpsimd.tensor_scalar_mul(rhs_v[i][:, BV:], big[:, BV:],
                                            maskt[i])

            # ---- indicator matmuls over t (both M-tiles accumulate into
            # zero-padded [36, *] psums; rows 0:4 = i0, 32:36 = i1) ----
            ps_v = psB.tile([36, 2 * BV], F32, tag="v")
            ps_j = psB.tile([36, 832], F32, tag="j")
            for i in range(NMT):
                ind = ind36[:, 36 * i:36 * (i + 1)]
                for lo, hi in NSL3:
                    nc.tensor.matmul(ps_v[:, lo:hi], ind, rhs_v[i][:, lo:hi],
                                     start=(i == 0), stop=(i == NMT - 1))
                for lo, hi in NSLJ:
                    nc.tensor.matmul(ps_j[:, lo:hi], ind, esf[i][:, lo:hi],
                                     start=(i == 0), stop=(i == NMT - 1))

            # ---- vps2 path at [36, x] ----
            rdv = smalls.tile([36, BV], F32)
            nc.vector.reciprocal_approx_fast(rdv, ps_v[:36, BV:])
            v2t = smalls.tile([36, BV], F32)
            nc.vector.tensor_tensor(v2t, ps_v[:36, :BV], rdv, op=MUL)
            fe4 = bigp.tile([36, 2 * BV], F32)
            nc.scalar.activation(fe4[:, BV:], v2t, EXP, scale=TAU)
            nc.vector.tensor_tensor(fe4[:, :BV], fe4[:, BV:], ps_j[:36, :BV],
                                    op=MUL)
            ored = smalls.tile([36, 128], F32)
            nc.vector.reduce_sum(ored, fe4.rearrange("p (g v) -> p g v", v=V),
                                 axis=X)
            dd = smalls.tile([36, B], F32)
            nc.vector.tensor_tensor(dd, ored[:, B:], ps_j[:36, BV:], op=MUL)
            rdd = smalls.tile([36, B], F32)
            nc.vector.reciprocal_approx_fast(rdd, dd)
            outw = smalls.tile([36, B], F32)
            nc.vector.tensor_tensor(outw, ored[:, :B], rdd, op=MUL)
            nc.sync.dma_start(out=out_d[0:APB, :], in_=outw[0:APB, :])
            nc.sync.dma_start(out=out_d[APB:2 * APB, :], in_=outw[32:36, :])

    nc.compile()
    return nc


_NC_CACHE = None


def _get_program():
    global _NC_CACHE
    if _NC_CACHE is None:
        _NC_CACHE = _build_program()
    return _NC_CACHE


def _make_in_maps(text_feat, video_feat, text_mask):
    vT = np.ascontiguousarray(video_feat.reshape(BV, D).T)
    ident = np.eye(128, dtype=np.float32)
    # Columns 32i + p//T are the real block indicators; all other columns
    # get 1/32 so the dead psum rows stay finite (recip of 0 is NaN and
    # trips CoreSim's nonfinite check; the dead rows are never read).
    ind36 = np.full((128, 2 * 36), 1.0 / T, np.float32)
    for i in range(NMT):
        for p in range(128):
            for j in range(2 * 36):
                if j // 36 == i and j % 36 in (0, 1, 2, 3, 32, 33, 34, 35):
                    ind36[p, j] = 0.0
            ind36[p, 36 * i + 32 * i + p // T] = 1.0
    onesc = np.ones((128, 1), np.float32)
    in_maps = []
    for c in range(NCORES):
        tsl = text_feat[c * AL:(c + 1) * AL].reshape(AT, D)
        in_maps.append({
            "tT": np.ascontiguousarray(tsl.T),
            "vT": vT,
            "mask": text_mask[c * AL:(c + 1) * AL].reshape(AT, 1)
                    .astype(np.float32),
            "ident": ident,
            "ind36": ind36,
            "onesc": onesc,
        })
    return in_maps


def kernel(text_feat, video_feat, text_mask, _trace=False):
    text_feat = np.asarray(text_feat, dtype=np.float32)
    video_feat = np.asarray(video_feat, dtype=np.float32)
    text_mask = np.asarray(text_mask)
    nc = _get_program()
    in_maps = _make_in_maps(text_feat, video_feat, text_mask)
    res = run_bass_kernel_spmd(nc, in_maps, core_ids=list(range(NCORES)),
                               trace=_trace)
    out = np.concatenate([res.results[c]["out"] for c in range(NCORES)], axis=0)
    if _trace:
        kernel.last_exec_time_ns = res.exec_time_ns
        kernel.last_results = res
    return out


# revision 15
# speedup vs baseline: 1.5808x; 1.5808x over previous
"""Trainium2 Bass kernel for the DCM sparse-attention problem.

Math restructure: with t-hat/v-hat the row-normalized features and
S[(a,t),(b,v)] = <t-hat[a,t], v-hat[b,v]> the raw cosine logits, every
softmax-weighted aggregation in the reference collapses onto S:

  t2v[a,b,t] = sum_v vps1 * S            (free-dim group reduce)
  v2t[a,b,v] = sum_t tps1 * S            (mask-folded indicator matmul)
  out[a,b]   = sum_t tps2[t] sum_v vps2[v] S[t,v]

so the [A,B,T,D] intermediates never exist. The text-side mask rides in
the indicator matmul's stationary operand, so the softmax numerator /
denominator pair [E*S | E] is reused for both softmax axes. Each of the
8 cores handles 8 of the 64 text rows (A-sharding, video replicated).
"""

import sys

sys.path.insert(0, "/opt/trn_rl_repo")

import numpy as np

import concourse.bass as bass
import concourse.bacc as bacc
import concourse.tile as tile
from concourse import mybir
from concourse.bass_utils import run_bass_kernel_spmd

TAU = 100.0
A, T, B, V, D = 64, 32, 64, 12, 512
NCORES = 8
AL = A // NCORES          # a's per core = 8
AT = AL * T               # (a,t) rows per core = 256
BV = B * V                # (b,v) cols = 768
NMT = AT // 128           # M-tiles over (a,t) = 2
NKT = D // 128            # K-tiles over d = 4
APB = 128 // T            # a's per M-tile = 4
F32 = mybir.dt.float32
EXP = mybir.ActivationFunctionType.Exp
SQUARE = mybir.ActivationFunctionType.Square
SQRT = mybir.ActivationFunctionType.Sqrt
MUL = mybir.AluOpType.mult
X = mybir.AxisListType.X
NSL = [(0, 512), (512, 768)]                   # bank-aligned slices of 768
NSL3 = [(0, 512), (512, 1024), (1024, 1536)]   # ... of 1536


def _build_program():
    nc = bacc.Bacc("TRN2", target_bir_lowering=False)

    tT_d = nc.declare_dram_parameter("tT", [D, AT], F32, isOutput=False)
    vT_d = nc.declare_dram_parameter("vT", [D, BV], F32, isOutput=False)
    mask_d = nc.declare_dram_parameter("mask", [AT, 1], F32, isOutput=False)
    ident_d = nc.declare_dram_parameter("ident", [128, 128], F32, isOutput=False)
    ind36_d = nc.declare_dram_parameter("ind36", [128, 2 * 36], F32, isOutput=False)
    indW_d = nc.declare_dram_parameter("indW", [36, 2 * 128], F32, isOutput=False)
    onesc_d = nc.declare_dram_parameter("onesc", [128, 1], F32, isOutput=False)
    out_d = nc.declare_dram_parameter("out", [AL, B], F32, isOutput=True)

    with tile.TileContext(nc) as tc:
        with (
            tc.tile_pool(name="consts", bufs=1) as consts,
            tc.tile_pool(name="inputs", bufs=1) as inputs,
            tc.tile_pool(name="sq", bufs=3) as sqp,
            tc.tile_pool(name="big", bufs=1) as bigp,
            tc.tile_pool(name="smalls", bufs=1) as smalls,
            tc.tile_pool(name="psA", bufs=1, space="PSUM") as psA,
            tc.tile_pool(name="psB", bufs=1, space="PSUM") as psB,
        ):
            # ---- constants / small inputs ----
            ident = consts.tile([128, 128], F32)
            nc.sync.dma_start(out=ident, in_=ident_d[:, :])
            ind36 = consts.tile([128, 2 * 36], F32)
            nc.sync.dma_start(out=ind36, in_=ind36_d[:, :])
            indW = consts.tile([36, 2 * 128], F32)
            nc.sync.dma_start(out=indW, in_=indW_d[:, :])
            onesc = consts.tile([128, 1], F32)
            nc.sync.dma_start(out=onesc, in_=onesc_d[:, :])
            maskt = [consts.tile([128, 1], F32, name=f"maskt{i}") for i in range(NMT)]
            tau_m = [consts.tile([128, 1], F32, name=f"tau_m{i}") for i in range(NMT)]
            ind36m = [consts.tile([128, 36], F32, name=f"ind36m{i}")
                      for i in range(NMT)]
            for i in range(NMT):
                nc.sync.dma_start(out=maskt[i], in_=mask_d[128 * i:128 * (i + 1), :])
                nc.vector.tensor_scalar_mul(tau_m[i], maskt[i], TAU)
                nc.vector.tensor_scalar_mul(ind36m[i],
                                            ind36[:, 36 * i:36 * (i + 1)],
                                            maskt[i])

            # ---- main inputs ([d, row] layouts); vT first (longest chain) ----
            vT = [inputs.tile([128, BV], F32, name=f"vT{k}") for k in range(NKT)]
            tT = [inputs.tile([128, AT], F32, name=f"tT{k}") for k in range(NKT)]
            for k in range(NKT):
                nc.sync.dma_start(out=vT[k], in_=vT_d[128 * k:128 * (k + 1), :])
            for k in range(NKT):
                nc.sync.dma_start(out=tT[k], in_=tT_d[128 * k:128 * (k + 1), :])

            # ---- norms: ACT squares + ones-matmul column sums ----
            ps_ssv = psB.tile([1, BV], F32, tag="jv")
            ps_sst = psB.tile([1, AT], F32, tag="j")
            for k in range(NKT):
                sqv = sqp.tile([128, BV], F32, tag="sq", name=f"sqv{k}")
                nc.scalar.activation(sqv, vT[k], SQUARE)
                for lo, hi in NSL:
                    nc.tensor.matmul(ps_ssv[:, lo:hi], onesc, sqv[:, lo:hi],
                                     start=(k == 0), stop=(k == NKT - 1))
            for k in range(NKT):
                sqt = sqp.tile([128, AT], F32, tag="sqt", name=f"sqt{k}")
                nc.scalar.activation(sqt, tT[k], SQUARE)
                nc.tensor.matmul(ps_sst, onesc, sqt,
                                 start=(k == 0), stop=(k == NKT - 1))

            # rv chain: sqrt (skinny) -> broadcast -> wide approx reciprocal
            nv_row = smalls.tile([1, BV], F32)
            nc.scalar.activation(nv_row, ps_ssv, SQRT)
            nv_bc = bigp.tile([128, BV], F32)
            nc.gpsimd.partition_broadcast(nv_bc, nv_row, channels=128)
            rv_bc = bigp.tile([128, BV], F32)
            nc.vector.reciprocal_approx_fast(rv_bc, nv_bc)

            # r_t: sqrt of norm row, transpose to per-partition column, recip
            r_t = [smalls.tile([128, 1], F32, name=f"r_t{i}") for i in range(NMT)]
            nt_row = smalls.tile([1, AT], F32)
            nc.scalar.activation(nt_row, ps_sst, SQRT)
            for i in range(NMT):
                ps_tr = psB.tile([128, 1], F32, tag="j", name=f"ps_tr{i}")
                nc.tensor.transpose(ps_tr, nt_row[:, 128 * i:128 * (i + 1)],
                                    ident[0:1, 0:1])
                nc.vector.reciprocal_approx_fast(r_t[i], ps_tr)

            # ---- S matmuls + per-M-tile softmax prep ----
            big = [bigp.tile([128, 2 * BV], F32, name=f"big{i}") for i in range(NMT)]
            rhs_f = [smalls.tile([128, 128], F32, name=f"rhs_f{i}")
                     for i in range(NMT)]
            sp = [bigp.tile([128, BV], F32, name=f"sp{i}") for i in range(NMT)]
            for i in range(NMT):
                ps_s = psA.tile([128, BV], F32, tag="s")
                for lo, hi in NSL:
                    for k in range(NKT):
                        nc.tensor.matmul(
                            ps_s[:, lo:hi],
                            tT[k][:, 128 * i:128 * (i + 1)],
                            vT[k][:, lo:hi],
                            start=(k == 0), stop=(k == NKT - 1))
                # Sp = ps_s * r_t (per-row) * rv (per-col)
                nc.vector.scalar_tensor_tensor(sp[i], ps_s, r_t[i], rv_bc,
                                               op0=MUL, op1=MUL)
                # big = [ES | E]; E = exp(tau*m*Sp)
                nc.scalar.activation(big[i][:, BV:], sp[i], EXP,
                                     scale=tau_m[i][:, :])
                nc.vector.tensor_tensor(big[i][:, :BV], big[i][:, BV:], sp[i],
                                        op=MUL)
                # t2v = groupsum(ES)/groupsum(E); E3 = exp(tau*t2v)
                red = smalls.tile([128, 128], F32, name=f"red{i}")
                nc.vector.reduce_sum(red,
                                     big[i].rearrange("p (g v) -> p g v", v=V),
                                     axis=X)
                rdn = smalls.tile([128, B], F32, name=f"rdn{i}")
                nc.vector.reciprocal_approx_fast(rdn, red[:, B:])
                t2v = smalls.tile([128, B], F32, name=f"t2v{i}")
                nc.vector.tensor_tensor(t2v, red[:, :B], rdn, op=MUL)
                nc.scalar.activation(rhs_f[i][:, B:], t2v, EXP, scale=TAU)

            # ---- v2t: mask-folded indicator matmul over t; rhs is [ES | E]
            # (rows 0:4 = M-tile 0, rows 32:36 = M-tile 1; other rows are
            # finite garbage via the 1/T dead-column trick) ----
            ps_v = psB.tile([36, 2 * BV], F32, tag="v")
            for i in range(NMT):
                for lo, hi in NSL3:
                    nc.tensor.matmul(ps_v[:, lo:hi], ind36m[i], big[i][:, lo:hi],
                                     start=(i == 0), stop=(i == NMT - 1))

            # ---- vps2 path at [36, x] ----
            rdv = smalls.tile([36, BV], F32)
            nc.vector.reciprocal_approx_fast(rdv, ps_v[:36, BV:])
            fe4 = bigp.tile([36, BV], F32)
            v2t = smalls.tile([36, BV], F32)
            nc.vector.tensor_tensor(v2t, ps_v[:36, :BV], rdv, op=MUL)
            nc.scalar.activation(fe4, v2t, EXP, scale=TAU)
            d4 = smalls.tile([36, B], F32)
            nc.vector.reduce_sum(d4, fe4.rearrange("p (g v) -> p g v", v=V),
                                 axis=X)

            # ---- broadcast E4 back over t-rows (PE), weight Sp, group-sum,
            # then the final indicator matmul computes sum_t ----
            for i in range(NMT):
                ps_w = psA.tile([128, BV], F32, tag="s", name=f"ps_w{i}")
                for lo, hi in NSL:
                    nc.tensor.matmul(ps_w[:, lo:hi],
                                     indW[:, 128 * i:128 * (i + 1)],
                                     fe4[:, lo:hi], start=True, stop=True)
                w4s = sqp.tile([128, BV], F32, tag="sq", name=f"w4s{i}")
                nc.vector.tensor_tensor(w4s, ps_w, sp[i], op=MUL)
                hun = smalls.tile([128, B], F32, name=f"hun{i}")
                nc.vector.reduce_sum(hun,
                                     w4s.rearrange("p (g v) -> p g v", v=V),
                                     axis=X)
                nc.vector.tensor_tensor(rhs_f[i][:, :B], rhs_f[i][:, B:], hun,
                                        op=MUL)

            ps_o = psB.tile([36, 128], F32, tag="j")
            for i in range(NMT):
                nc.tensor.matmul(ps_o, ind36[:, 36 * i:36 * (i + 1)], rhs_f[i],
                                 start=(i == 0), stop=(i == NMT - 1))
            dd = smalls.tile([36, B], F32)
            nc.vector.tensor_tensor(dd, ps_o[:36, B:], d4, op=MUL)
            rdd = smalls.tile([36, B], F32)
            nc.vector.reciprocal_approx_fast(rdd, dd)
            outw = smalls.tile([36, B], F32)
            nc.vector.tensor_tensor(outw, ps_o[:36, :B], rdd, op=MUL)
            nc.sync.dma_start(out=out_d[0:APB, :], in_=outw[0:APB, :])
            nc.sync.dma_start(out=out_d[APB:2 * APB, :], in_=outw[32:36, :])

    nc.compile()
    return nc


_NC_CACHE = None


def _get_program():
    global _NC_CACHE
    if _NC_CACHE is None:
        _NC_CACHE = _build_program()
    return _NC_CACHE


def _make_in_maps(text_feat, video_feat, text_mask):
    vT = np.ascontiguousarray(video_feat.reshape(BV, D).T)
    ident = np.eye(128, dtype=np.float32)
    # ind36 slice i: column 32i + p//T is the block indicator; the other
    # M-tile's real rows get 0; dead rows get 1/T so every psum row stays
    # finite through the reciprocal (dead rows are never read back).
    real = (0, 1, 2, 3, 32, 33, 34, 35)
    ind36 = np.full((128, 2 * 36), 1.0 / T, np.float32)
    for i in range(NMT):
        for c in real:
            ind36[:, 36 * i + c] = 0.0
        for p in range(128):
            ind36[p, 36 * i + 32 * i + p // T] = 1.0
    # indW slice i: [36, 128] with indW[r, p] = (r == 32i + p//T), so the
    # broadcast matmul copies E4 row 32i+p//T into partition p.
    indW = np.zeros((36, 2 * 128), np.float32)
    for i in range(NMT):
        for p in range(128):
            indW[32 * i + p // T, 128 * i + p] = 1.0
    onesc = np.ones((128, 1), np.float32)
    in_maps = []
    for c in range(NCORES):
        tsl = text_feat[c * AL:(c + 1) * AL].reshape(AT, D)
        in_maps.append({
            "tT": np.ascontiguousarray(tsl.T),
            "vT": vT,
            "mask": text_mask[c * AL:(c + 1) * AL].reshape(AT, 1)
                    .astype(np.float32),
            "ident": ident,
            "ind36": ind36,
            "indW": indW,
            "onesc": onesc,
        })
    return in_maps


def kernel(text_feat, video_feat, text_mask, _trace=False):
    text_feat = np.asarray(text_feat, dtype=np.float32)
    video_feat = np.asarray(video_feat, dtype=np.float32)
    text_mask = np.asarray(text_mask)
    nc = _get_program()
    in_maps = _make_in_maps(text_feat, video_feat, text_mask)
    res = run_bass_kernel_spmd(nc, in_maps, core_ids=list(range(NCORES)),
                               trace=_trace)
    out = np.concatenate([res.results[c]["out"] for c in range(NCORES)], axis=0)
    if _trace:
        kernel.last_exec_time_ns = res.exec_time_ns
        kernel.last_results = res
    return out


# revision 18
# speedup vs baseline: 1.5995x; 1.0118x over previous
"""Trainium2 Bass kernel for the DCM sparse-attention problem.

Math restructure: with t-hat/v-hat the row-normalized features and
S[(a,t),(b,v)] = <t-hat[a,t], v-hat[b,v]> the raw cosine logits, every
softmax-weighted aggregation in the reference collapses onto S:

  t2v[a,b,t] = sum_v vps1 * S            (free-dim group reduce)
  v2t[a,b,v] = sum_t tps1 * S            (mask-folded indicator matmul)
  out[a,b]   = sum_t tps2[t] sum_v vps2[v] S[t,v]

so the [A,B,T,D] intermediates never exist. The text-side mask rides in
the indicator matmul's stationary operand, so the softmax numerator /
denominator pair [E*S | E] is reused for both softmax axes. Each of the
8 cores handles 8 of the 64 text rows (A-sharding, video replicated).
"""

import sys

sys.path.insert(0, "/opt/trn_rl_repo")

import ml_dtypes
import numpy as np

import concourse.bass as bass
import concourse.bacc as bacc
import concourse.tile as tile
from concourse import mybir
from concourse.bass_utils import run_bass_kernel_spmd

TAU = 100.0
A, T, B, V, D = 64, 32, 64, 12, 512
NCORES = 8
AL = A // NCORES          # a's per core = 8
AT = AL * T               # (a,t) rows per core = 256
BV = B * V                # (b,v) cols = 768
NMT = AT // 128           # M-tiles over (a,t) = 2
NKT = D // 128            # K-tiles over d = 4
APB = 128 // T            # a's per M-tile = 4
F32 = mybir.dt.float32
BF16 = mybir.dt.bfloat16
EXP = mybir.ActivationFunctionType.Exp
SQUARE = mybir.ActivationFunctionType.Square
SQRT = mybir.ActivationFunctionType.Sqrt
MUL = mybir.AluOpType.mult
X = mybir.AxisListType.X
NSL = [(0, 512), (512, 768)]                   # bank-aligned slices of 768
NSL3 = [(0, 512), (512, 1024), (1024, 1536)]   # ... of 1536


def _build_program():
    nc = bacc.Bacc("TRN2", target_bir_lowering=False)

    tT_d = nc.declare_dram_parameter("tT", [D, AT], F32, isOutput=False)
    vT_d = nc.declare_dram_parameter("vT", [D, BV], F32, isOutput=False)
    mask_d = nc.declare_dram_parameter("mask", [AT, 1], F32, isOutput=False)
    ident_d = nc.declare_dram_parameter("ident", [128, 128], F32, isOutput=False)
    ind36_d = nc.declare_dram_parameter("ind36", [128, 2 * 36], F32, isOutput=False)
    indW_d = nc.declare_dram_parameter("indW", [36, 2 * 128], BF16, isOutput=False)
    onesc_d = nc.declare_dram_parameter("onesc", [128, 1], BF16, isOutput=False)
    out_d = nc.declare_dram_parameter("out", [AL, B], F32, isOutput=True)

    with tile.TileContext(nc) as tc:
        with (
            tc.tile_pool(name="consts", bufs=1) as consts,
            tc.tile_pool(name="inputs", bufs=1) as inputs,
            tc.tile_pool(name="sq", bufs=3) as sqp,
            tc.tile_pool(name="big", bufs=1) as bigp,
            tc.tile_pool(name="smalls", bufs=1) as smalls,
            tc.tile_pool(name="psA", bufs=2, space="PSUM") as psA,
            tc.tile_pool(name="psB", bufs=1, space="PSUM") as psB,
        ):
            # ---- constants / small inputs ----
            ident = consts.tile([128, 128], F32)
            nc.sync.dma_start(out=ident, in_=ident_d[:, :])
            ind36 = consts.tile([128, 2 * 36], F32)
            nc.sync.dma_start(out=ind36, in_=ind36_d[:, :])
            indW = consts.tile([36, 2 * 128], BF16)
            nc.sync.dma_start(out=indW, in_=indW_d[:, :])
            onesc = consts.tile([128, 1], BF16)
            nc.sync.dma_start(out=onesc, in_=onesc_d[:, :])
            maskt = [consts.tile([128, 1], F32, name=f"maskt{i}") for i in range(NMT)]
            tau_m = [consts.tile([128, 1], F32, name=f"tau_m{i}") for i in range(NMT)]
            ind36m = [consts.tile([128, 36], F32, name=f"ind36m{i}")
                      for i in range(NMT)]
            for i in range(NMT):
                nc.sync.dma_start(out=maskt[i], in_=mask_d[128 * i:128 * (i + 1), :])
                nc.vector.tensor_scalar_mul(tau_m[i], maskt[i], TAU)
                nc.vector.tensor_scalar_mul(ind36m[i],
                                            ind36[:, 36 * i:36 * (i + 1)],
                                            maskt[i])

            # ---- main inputs ([d, row] layouts); vT first (longest chain) ----
            vT = [inputs.tile([128, BV], F32, name=f"vT{k}") for k in range(NKT)]
            tT = [inputs.tile([128, AT], F32, name=f"tT{k}") for k in range(NKT)]
            for k in range(NKT):
                nc.sync.dma_start(out=vT[k], in_=vT_d[128 * k:128 * (k + 1), :])
            for k in range(NKT):
                nc.sync.dma_start(out=tT[k], in_=tT_d[128 * k:128 * (k + 1), :])

            # ---- S matmuls first: PE starts as soon as DMA lands and
            # stays dense long enough to warm the HAM throttle ----
            ps_s = [psA.tile([128, BV], F32, tag="s", name=f"ps_s{i}")
                    for i in range(NMT)]
            for i in range(NMT):
                for lo, hi in NSL:
                    for k in range(NKT):
                        nc.tensor.matmul(
                            ps_s[i][:, lo:hi],
                            tT[k][:, 128 * i:128 * (i + 1)],
                            vT[k][:, lo:hi],
                            start=(k == 0), stop=(k == NKT - 1))

            # ---- norms: ACT squares + ones-matmul column sums ----
            ps_ssv = psB.tile([1, BV], F32, tag="v")
            ps_sst = psB.tile([1, AT], F32, tag="j")
            for k in range(NKT):
                sqv = sqp.tile([128, BV], BF16, tag="sqv", name=f"sqv{k}")
                nc.scalar.activation(sqv, vT[k], SQUARE)
                for lo, hi in NSL:
                    nc.tensor.matmul(ps_ssv[:, lo:hi], onesc, sqv[:, lo:hi],
                                     start=(k == 0), stop=(k == NKT - 1))
            for k in range(NKT):
                sqt = sqp.tile([128, AT], BF16, tag="sqt", name=f"sqt{k}")
                nc.scalar.activation(sqt, tT[k], SQUARE)
                nc.tensor.matmul(ps_sst, onesc, sqt,
                                 start=(k == 0), stop=(k == NKT - 1))

            # rv chain: sqrt (skinny) -> broadcast -> wide approx reciprocal
            nv_row = smalls.tile([1, BV], F32)
            nc.scalar.activation(nv_row, ps_ssv, SQRT)
            nv_bc = bigp.tile([128, BV], F32)
            nc.gpsimd.partition_broadcast(nv_bc, nv_row, channels=128)
            rv_bc = bigp.tile([128, BV], F32)
            nc.vector.reciprocal_approx_fast(rv_bc, nv_bc)

            # r_t: sqrt of norm row, transpose to per-partition column, recip
            r_t = [smalls.tile([128, 1], F32, name=f"r_t{i}") for i in range(NMT)]
            nt_row = smalls.tile([1, AT], F32)
            nc.scalar.activation(nt_row, ps_sst, SQRT)
            for i in range(NMT):
                ps_tr = psB.tile([128, 1], F32, tag="j", name=f"ps_tr{i}")
                nc.tensor.transpose(ps_tr, nt_row[:, 128 * i:128 * (i + 1)],
                                    ident[0:1, 0:1])
                nc.vector.reciprocal_approx_fast(r_t[i], ps_tr)

            # ---- S matmuls + per-M-tile softmax prep ----
            big = [bigp.tile([128, 2 * BV], F32, name=f"big{i}") for i in range(NMT)]
            rhs_f = [smalls.tile([128, 128], F32, name=f"rhs_f{i}")
                     for i in range(NMT)]
            sp = [bigp.tile([128, BV], F32, name=f"sp{i}") for i in range(NMT)]
            for i in range(NMT):
                # Sp = ps_s * r_t (per-row) * rv (per-col)
                nc.vector.scalar_tensor_tensor(sp[i], ps_s[i], r_t[i], rv_bc,
                                               op0=MUL, op1=MUL)
                # big = [ES | E]; E = exp(tau*m*Sp)
                nc.scalar.activation(big[i][:, BV:], sp[i], EXP,
                                     scale=tau_m[i][:, :])
                nc.vector.tensor_tensor(big[i][:, :BV], big[i][:, BV:], sp[i],
                                        op=MUL)
                # t2v = groupsum(ES)/groupsum(E); E3 = exp(tau*t2v)
                red = smalls.tile([128, 128], F32, name=f"red{i}")
                nc.vector.reduce_sum(red,
                                     big[i].rearrange("p (g v) -> p g v", v=V),
                                     axis=X)
                rdn = smalls.tile([128, B], F32, name=f"rdn{i}")
                nc.vector.reciprocal_approx_fast(rdn, red[:, B:])
                t2v = smalls.tile([128, B], F32, name=f"t2v{i}")
                nc.vector.tensor_tensor(t2v, red[:, :B], rdn, op=MUL)
                nc.scalar.activation(rhs_f[i][:, B:], t2v, EXP, scale=TAU)

            # ---- v2t: mask-folded indicator matmul over t; rhs is [ES | E]
            # (rows 0:4 = M-tile 0, rows 32:36 = M-tile 1; other rows are
            # finite garbage via the 1/T dead-column trick) ----
            ps_v = psB.tile([36, 2 * BV], F32, tag="v")
            for i in range(NMT):
                for lo, hi in NSL3:
                    nc.tensor.matmul(ps_v[:, lo:hi], ind36m[i], big[i][:, lo:hi],
                                     start=(i == 0), stop=(i == NMT - 1))

            # ---- vps2 path at [36, x] ----
            rdv = smalls.tile([36, BV], F32)
            nc.vector.reciprocal_approx_fast(rdv, ps_v[:36, BV:])
            fe4 = bigp.tile([36, BV], BF16)
            v2t = smalls.tile([36, BV], F32)
            nc.vector.tensor_tensor(v2t, ps_v[:36, :BV], rdv, op=MUL)
            nc.scalar.activation(fe4, v2t, EXP, scale=TAU)
            d4 = smalls.tile([36, B], F32)
            nc.vector.reduce_sum(d4, fe4.rearrange("p (g v) -> p g v", v=V),
                                 axis=X)

            # ---- broadcast E4 back over t-rows (PE), weight Sp, group-sum,
            # then the final indicator matmul computes sum_t ----
            for i in range(NMT):
                ps_w = psA.tile([128, BV], F32, tag="s", name=f"ps_w{i}")
                for lo, hi in NSL:
                    nc.tensor.matmul(ps_w[:, lo:hi],
                                     indW[:, 128 * i:128 * (i + 1)],
                                     fe4[:, lo:hi], start=True, stop=True)
                w4s = sqp.tile([128, BV], F32, tag="sq", name=f"w4s{i}")
                nc.vector.tensor_tensor(w4s, ps_w, sp[i], op=MUL)
                hun = smalls.tile([128, B], F32, name=f"hun{i}")
                nc.vector.reduce_sum(hun,
                                     w4s.rearrange("p (g v) -> p g v", v=V),
                                     axis=X)
                nc.vector.tensor_tensor(rhs_f[i][:, :B], rhs_f[i][:, B:], hun,
                                        op=MUL)

            ps_o = psB.tile([36, 128], F32, tag="j")
            for i in range(NMT):
                nc.tensor.matmul(ps_o, ind36[:, 36 * i:36 * (i + 1)], rhs_f[i],
                                 start=(i == 0), stop=(i == NMT - 1))
            dd = smalls.tile([36, B], F32)
            nc.vector.tensor_tensor(dd, ps_o[:36, B:], d4, op=MUL)
            rdd = smalls.tile([36, B], F32)
            nc.vector.reciprocal_approx_fast(rdd, dd)
            outw = smalls.tile([36, B], F32)
            nc.vector.tensor_tensor(outw, ps_o[:36, :B], rdd, op=MUL)
            nc.sync.dma_start(out=out_d[0:APB, :], in_=outw[0:APB, :])
            nc.sync.dma_start(out=out_d[APB:2 * APB, :], in_=outw[32:36, :])

    nc.compile()
    return nc


_NC_CACHE = None


def _get_program():
    global _NC_CACHE
    if _NC_CACHE is None:
        _NC_CACHE = _build_program()
    return _NC_CACHE


def _make_in_maps(text_feat, video_feat, text_mask):
    vT = np.ascontiguousarray(video_feat.reshape(BV, D).T)
    ident = np.eye(128, dtype=np.float32)
    # ind36 slice i: column 32i + p//T is the block indicator; the other
    # M-tile's real rows get 0; dead rows get 1/T so every psum row stays
    # finite through the reciprocal (dead rows are never read back).
    real = (0, 1, 2, 3, 32, 33, 34, 35)
    ind36 = np.full((128, 2 * 36), 1.0 / T, np.float32)
    for i in range(NMT):
        for c in real:
            ind36[:, 36 * i + c] = 0.0
        for p in range(128):
            ind36[p, 36 * i + 32 * i + p // T] = 1.0
    # indW slice i: [36, 128] with indW[r, p] = (r == 32i + p//T), so the
    # broadcast matmul copies E4 row 32i+p//T into partition p.
    indW = np.zeros((36, 2 * 128), ml_dtypes.bfloat16)
    for i in range(NMT):
        for p in range(128):
            indW[32 * i + p // T, 128 * i + p] = 1.0
    onesc = np.ones((128, 1), ml_dtypes.bfloat16)
    in_maps = []
    for c in range(NCORES):
        tsl = text_feat[c * AL:(c + 1) * AL].reshape(AT, D)
        in_maps.append({
            "tT": np.ascontiguousarray(tsl.T),
            "vT": vT,
            "mask": text_mask[c * AL:(c + 1) * AL].reshape(AT, 1)
                    .astype(np.float32),
            "ident": ident,
            "ind36": ind36,
            "indW": indW,
            "onesc": onesc,
        })
    return in_maps


def kernel(text_feat, video_feat, text_mask, _trace=False):
    text_feat = np.asarray(text_feat, dtype=np.float32)
    video_feat = np.asarray(video_feat, dtype=np.float32)
    text_mask = np.asarray(text_mask)
    nc = _get_program()
    in_maps = _make_in_maps(text_feat, video_feat, text_mask)
    res = run_bass_kernel_spmd(nc, in_maps, core_ids=list(range(NCORES)),
                               trace=_trace)
    out = np.concatenate([res.results[c]["out"] for c in range(NCORES)], axis=0)
    if _trace:
        kernel.last_exec_time_ns = res.exec_time_ns
        kernel.last_results = res
    return out
